# revision 9
# baseline (speedup 1.0000x reference)
"""HardAttention kernel for Trainium2 (8 NeuronCores, Bass/Tile).

reference:
    scores = einsum("btd,bcsd->btcs", xs, ys)   # (B,Tx,C,Ty)
    out    = scores.max(-1).sum(1)              # (B,C)

Shapes: B=16, Tx=128, C=64, Ty=128, d=768.

Strategy (v4, fp8 + DoubleRow, tuned pipeline):
  - Data-parallel over B: core i handles batches [2i, 2i+2).
  - Host pre-casts both operands to fp8 e4m3 (inputs are N(0,1); max |v|
    ~5.4, far below the 240 clip; measured end-to-end max rel err 0.46%
    vs the 2% gate) and lays them out d-major, pre-paired for DoubleRow:
        d = 256*kk + 128*j + p   (kk in 0..2, j in 0..1, p in 0..127)
        xsP[p, b, j, kk, t]    = xs[b, t, d]      (128, B, 2, 3, Tx)
        ysP[b, p, j, c, kk, s] = ys[b, c, s, d]   (B, 128, 2, C, 3, Ty)
  - Slab DMAs alternate between the two HWDGE rings (sync + scalar) so
    descriptor generation pipelines.
  - ~16 throwaway warm-up matmuls right after xs lands keep the PE HAM
    at K=8/8 before the first real slab arrives.
  - Per (b, quarter-of-64-candidates): one 1.57 MB HWDGE DMA, then
    DoubleRow matmuls (K=256, N=512) accumulating into 4 PSUM banks,
    kk-outer so 4 matmuls share each LDWEIGHTS target; DVE reduce_max
    over Ty into an SBUF tile m[t, c]; per-batch ones-vector fp32 matmul
    contracts the partition axis (sum over t) -> out[b, c] so batch 0's
    output path completes mid-kernel.
"""

import numpy as np

B, TX, C, TY, D = 16, 128, 64, 128, 768
N_CORES = 8
BPC = B // N_CORES          # batches per core = 2
KK = D // 256               # DoubleRow contraction chunks = 3
QC = 16                     # candidates per DMA slab
NQ = C // QC                # slabs per batch = 4
G = 4                       # candidates per matmul (N = G*TY = 512)
N_WARM = 12                 # PE warm-up matmuls (bridge until first slab lands)
# Per-batch slab sizes (candidates per DMA). A small first slab lets real
# matmuls start ~5us earlier; the rest use 16 for DMA efficiency.
SLABS = {0: [4, 16, 16, 16, 12], 1: [16, 16, 16, 16]}

_CACHE = {}


def _build():
    import concourse.bass as bass
    import concourse.mybir as mybir
    import concourse.tile as tile
    from concourse import bacc

    fp8 = mybir.dt.float8e4
    f32 = mybir.dt.float32
    DR = mybir.MatmulPerfMode.DoubleRow

    nc = bacc.Bacc(
        "TRN2",
        target_bir_lowering=False,
        debug=False,
        num_devices=N_CORES,
    )

    xs_ap = nc.dram_tensor(
        "xsP", (128, BPC, 2, KK, TX), fp8, kind="ExternalInput"
    ).ap()
    ys_ap = nc.dram_tensor(
        "ysP", (BPC, 128, 2, C, KK, TY), fp8, kind="ExternalInput"
    ).ap()
    out_ap = nc.dram_tensor("out", (BPC, C), f32, kind="ExternalOutput").ap()

    with tile.TileContext(nc) as tc:
        with (
            tc.tile_pool(name="xt", bufs=1) as xpool,
            tc.tile_pool(name="yt", bufs=8) as ypool,
            tc.tile_pool(name="mt", bufs=1) as mpool,
            tc.tile_pool(name="ones", bufs=1) as opool,
            tc.tile_pool(name="osb", bufs=2) as obpool,
            tc.tile_pool(name="ps", bufs=7, space="PSUM") as pspool,
            tc.tile_pool(name="pso", bufs=1, space="PSUM") as psopool,
        ):
            # All of xsP for this core: (p, b, j, kk, t) — 1.5 KB/partition
            xt = xpool.tile([128, BPC, 2, KK, TX], fp8)
            nc.sync.dma_start(xt[:], xs_ap)

            ones = opool.tile([128, 1], f32)
            nc.any.memset(ones[:], 1.0)

            # max_s scores: [t, (b, c)]
            m_all = mpool.tile([128, BPC, C], f32)

            # PE warm-up: throwaway DoubleRow matmuls on xs data so the HAM
            # clock-gate reaches K=8/8 before the first real slab lands.
            warm = psopool.tile([128, TX], f32, tag="pso", name="warm")
            for w in range(N_WARM):
                nc.tensor.matmul(
                    warm[:],
                    lhsT=xt[:, 0, :, w % KK, :],
                    rhs=xt[:, 0, :, (w + 1) % KK, :],
                    start=True,
                    stop=True,
                    perf_mode=DR,
                )

            dma_engines = [nc.scalar, nc.sync]
            si = 0
            for b in range(BPC):
                c_base = 0
                for q, qc in enumerate(SLABS[b]):
                    # slab: (p, j, c_in_slab, kk, s)
                    yt = ypool.tile(
                        [128, 2, qc, KK, TY], fp8, name=f"yt_{b}_{q}", tag="yt"
                    )
                    dma_engines[si % 2].dma_start(
                        yt[:], ys_ap[b, :, :, c_base : c_base + qc, :, :]
                    )
                    si += 1
                    ngr = qc // G
                    psums = [
                        pspool.tile(
                            [128, G, TY], f32, name=f"ps_{b}_{q}_{g}", tag="ps"
                        )
                        for g in range(ngr)
                    ]
                    # g-outer: each bank finishes early so its reduce
                    # overlaps the next bank's matmuls and frees PSUM early.
                    for g in range(ngr):
                        for kk in range(KK):
                            nc.tensor.matmul(
                                psums[g][:],
                                lhsT=xt[:, b, :, kk, :],
                                rhs=yt[:, :, g * G : (g + 1) * G, kk, :],
                                start=(kk == 0),
                                stop=(kk == KK - 1),
                                perf_mode=DR,
                            )
                    for g in range(ngr):
                        c0 = c_base + g * G
                        nc.vector.reduce_max(
                            m_all[:, b, c0 : c0 + G],
                            psums[g][:],
                            axis=mybir.AxisListType.X,
                        )
                    c_base += qc
                # sum over t (partition axis) via ones-vector matmul, fp32.
                # Per batch so batch 0's output completes mid-kernel.
                out_ps = psopool.tile([1, C], f32, tag="pso", name=f"out_ps{b}")
                nc.tensor.matmul(
                    out_ps[:], lhsT=ones[:], rhs=m_all[:, b, :], start=True, stop=True
                )
                osb = obpool.tile([1, C], f32, tag="osb")
                nc.vector.tensor_copy(osb[:], out_ps[:])
                nc.sync.dma_start(out_ap[b : b + 1, :], osb[:])

    nc.compile()
    return nc


def _get_nc():
    if "nc" not in _CACHE:
        _CACHE["nc"] = _build()
    return _CACHE["nc"]


def _prep(xs: np.ndarray, ys: np.ndarray):
    """Host-side layout: fp8 e4m3 cast + d-major DoubleRow-paired blocks."""
    import ml_dtypes

    fp8 = ml_dtypes.float8_e4m3
    xsb = np.asarray(xs, dtype=np.float32).astype(fp8)
    ysb = np.asarray(ys, dtype=np.float32).astype(fp8)
    # xsP[p, b, j, kk, t] = xs[b, t, 256kk+128j+p]
    xsP = np.ascontiguousarray(
        xsb.reshape(B, TX, KK, 2, 128).transpose(4, 0, 3, 2, 1)
    )
    # ysP[b, p, j, c, kk, s] = ys[b, c, s, 256kk+128j+p]
    ysP = np.ascontiguousarray(
        ysb.reshape(B, C, TY, KK, 2, 128).transpose(0, 5, 4, 1, 3, 2)
    )
    return xsP, ysP


def kernel(xs: np.ndarray, ys: np.ndarray) -> np.ndarray:
    from concourse.bass_utils import run_bass_kernel_spmd

    nc = _get_nc()
    xsP, ysP = _prep(xs, ys)
    in_maps = [
        {
            "xsP": np.ascontiguousarray(xsP[:, i * BPC : (i + 1) * BPC]),
            "ysP": ysP[i * BPC : (i + 1) * BPC],
        }
        for i in range(N_CORES)
    ]
    res = run_bass_kernel_spmd(nc, in_maps, core_ids=list(range(N_CORES)))
    _CACHE["last_result"] = res
    out = np.concatenate(
        [res.results[i]["out"] for i in range(N_CORES)], axis=0
    )
    return out.astype(np.float32)


# revision 11
# speedup vs baseline: 1.0464x; 1.0464x over previous
"""HardAttention kernel for Trainium2 (8 NeuronCores, Bass/Tile).

reference:
    scores = einsum("btd,bcsd->btcs", xs, ys)   # (B,Tx,C,Ty)
    out    = scores.max(-1).sum(1)              # (B,C)

Shapes: B=16, Tx=128, C=64, Ty=128, d=768.

Strategy (v4, fp8 + DoubleRow, tuned pipeline):
  - Data-parallel over B: core i handles batches [2i, 2i+2).
  - Host pre-casts both operands to fp8 e4m3 (inputs are N(0,1); max |v|
    ~5.4, far below the 240 clip; measured end-to-end max rel err 0.46%
    vs the 2% gate) and lays them out d-major, pre-paired for DoubleRow:
        d = 256*kk + 128*j + p   (kk in 0..2, j in 0..1, p in 0..127)
        xsP[p, b, j, kk, t]    = xs[b, t, d]      (128, B, 2, 3, Tx)
        ysP[b, p, j, c, kk, s] = ys[b, c, s, d]   (B, 128, 2, C, 3, Ty)
  - Slab DMAs alternate between the two HWDGE rings (sync + scalar) so
    descriptor generation pipelines.
  - ~16 throwaway warm-up matmuls right after xs lands keep the PE HAM
    at K=8/8 before the first real slab arrives.
  - Per (b, quarter-of-64-candidates): one 1.57 MB HWDGE DMA, then
    DoubleRow matmuls (K=256, N=512) accumulating into 4 PSUM banks,
    kk-outer so 4 matmuls share each LDWEIGHTS target; DVE reduce_max
    over Ty into an SBUF tile m[t, c]; per-batch ones-vector fp32 matmul
    contracts the partition axis (sum over t) -> out[b, c] so batch 0's
    output path completes mid-kernel.
"""

import numpy as np

B, TX, C, TY, D = 16, 128, 64, 128, 768
N_CORES = 8
BPC = B // N_CORES          # batches per core = 2
KK = D // 256               # DoubleRow contraction chunks = 3
QC = 16                     # candidates per DMA slab
NQ = C // QC                # slabs per batch = 4
G = 4                       # candidates per matmul (N = G*TY = 512)
N_WARM = 12                 # PE warm-up matmuls (bridge until first slab lands)
# Per-batch slab sizes (candidates per DMA). Small first slabs let real
# matmuls start ~5us earlier; a small final slab shortens the reduce tail.
SLABS = {0: [4, 8, 16, 16, 16, 4], 1: [16, 16, 16, 12, 4]}

_CACHE = {}


def _build():
    import concourse.bass as bass
    import concourse.mybir as mybir
    import concourse.tile as tile
    from concourse import bacc

    fp8 = mybir.dt.float8e4
    f32 = mybir.dt.float32
    DR = mybir.MatmulPerfMode.DoubleRow

    nc = bacc.Bacc(
        "TRN2",
        target_bir_lowering=False,
        debug=False,
        num_devices=N_CORES,
    )

    xs_ap = nc.dram_tensor(
        "xsP", (128, BPC, 2, KK, TX), fp8, kind="ExternalInput"
    ).ap()
    ys_ap = nc.dram_tensor(
        "ysP", (BPC, 128, 2, C, KK, TY), fp8, kind="ExternalInput"
    ).ap()
    out_ap = nc.dram_tensor("out", (BPC, C), f32, kind="ExternalOutput").ap()

    with tile.TileContext(nc) as tc:
        with (
            tc.tile_pool(name="xt", bufs=1) as xpool,
            tc.tile_pool(name="yt", bufs=8) as ypool,
            tc.tile_pool(name="mt", bufs=1) as mpool,
            tc.tile_pool(name="ones", bufs=1) as opool,
            tc.tile_pool(name="osb", bufs=2) as obpool,
            tc.tile_pool(name="ps", bufs=7, space="PSUM") as pspool,
            tc.tile_pool(name="pso", bufs=1, space="PSUM") as psopool,
        ):
            # All of xsP for this core: (p, b, j, kk, t) — 1.5 KB/partition
            xt = xpool.tile([128, BPC, 2, KK, TX], fp8)
            nc.sync.dma_start(xt[:], xs_ap)

            ones = opool.tile([128, 1], f32)
            nc.any.memset(ones[:], 1.0)

            # max_s scores: [t, (b, c)]
            m_all = mpool.tile([128, BPC, C], f32)

            # PE warm-up: throwaway DoubleRow matmuls on xs data so the HAM
            # clock-gate reaches K=8/8 before the first real slab lands.
            warm = psopool.tile([128, TX], f32, tag="pso", name="warm")
            for w in range(N_WARM):
                nc.tensor.matmul(
                    warm[:],
                    lhsT=xt[:, 0, :, w % KK, :],
                    rhs=xt[:, 0, :, (w + 1) % KK, :],
                    start=True,
                    stop=True,
                    perf_mode=DR,
                )

            # All slab DMAs on one HWDGE ring: transfers complete strictly in
            # issue order, so each slab lands as early as possible for the PE.
            for b in range(BPC):
                c_base = 0
                for q, qc in enumerate(SLABS[b]):
                    # slab: (p, j, c_in_slab, kk, s)
                    yt = ypool.tile(
                        [128, 2, qc, KK, TY], fp8, name=f"yt_{b}_{q}", tag="yt"
                    )
                    nc.sync.dma_start(
                        yt[:], ys_ap[b, :, :, c_base : c_base + qc, :, :]
                    )
                    ngr = qc // G
                    psums = [
                        pspool.tile(
                            [128, G, TY], f32, name=f"ps_{b}_{q}_{g}", tag="ps"
                        )
                        for g in range(ngr)
                    ]
                    # g-outer: each bank finishes early so its reduce
                    # overlaps the next bank's matmuls and frees PSUM early.
                    for g in range(ngr):
                        for kk in range(KK):
                            nc.tensor.matmul(
                                psums[g][:],
                                lhsT=xt[:, b, :, kk, :],
                                rhs=yt[:, :, g * G : (g + 1) * G, kk, :],
                                start=(kk == 0),
                                stop=(kk == KK - 1),
                                perf_mode=DR,
                            )
                    for g in range(ngr):
                        c0 = c_base + g * G
                        nc.vector.reduce_max(
                            m_all[:, b, c0 : c0 + G],
                            psums[g][:],
                            axis=mybir.AxisListType.X,
                        )
                    c_base += qc
                # sum over t (partition axis) via ones-vector matmul, fp32.
                # Per batch so batch 0's output completes mid-kernel.
                out_ps = psopool.tile([1, C], f32, tag="pso", name=f"out_ps{b}")
                nc.tensor.matmul(
                    out_ps[:], lhsT=ones[:], rhs=m_all[:, b, :], start=True, stop=True
                )
                osb = obpool.tile([1, C], f32, tag="osb")
                nc.vector.tensor_copy(osb[:], out_ps[:])
                nc.sync.dma_start(out_ap[b : b + 1, :], osb[:])

    nc.compile()
    return nc


def _get_nc():
    if "nc" not in _CACHE:
        _CACHE["nc"] = _build()
    return _CACHE["nc"]


def _prep(xs: np.ndarray, ys: np.ndarray):
    """Host-side layout: fp8 e4m3 cast + d-major DoubleRow-paired blocks."""
    import ml_dtypes

    fp8 = ml_dtypes.float8_e4m3
    xsb = np.asarray(xs, dtype=np.float32).astype(fp8)
    ysb = np.asarray(ys, dtype=np.float32).astype(fp8)
    # xsP[p, b, j, kk, t] = xs[b, t, 256kk+128j+p]
    xsP = np.ascontiguousarray(
        xsb.reshape(B, TX, KK, 2, 128).transpose(4, 0, 3, 2, 1)
    )
    # ysP[b, p, j, c, kk, s] = ys[b, c, s, 256kk+128j+p]
    ysP = np.ascontiguousarray(
        ysb.reshape(B, C, TY, KK, 2, 128).transpose(0, 5, 4, 1, 3, 2)
    )
    return xsP, ysP


def kernel(xs: np.ndarray, ys: np.ndarray) -> np.ndarray:
    from concourse.bass_utils import run_bass_kernel_spmd

    nc = _get_nc()
    xsP, ysP = _prep(xs, ys)
    in_maps = [
        {
            "xsP": np.ascontiguousarray(xsP[:, i * BPC : (i + 1) * BPC]),
            "ysP": ysP[i * BPC : (i + 1) * BPC],
        }
        for i in range(N_CORES)
    ]
    res = run_bass_kernel_spmd(nc, in_maps, core_ids=list(range(N_CORES)))
    _CACHE["last_result"] = res
    out = np.concatenate(
        [res.results[i]["out"] for i in range(N_CORES)], axis=0
    )
    return out.astype(np.float32)
